# revision 12
# baseline (speedup 1.0000x reference)
"""Trainium2 Bass kernel for nn_GridCellLoss (pairwise JSD + pdist loss).

Self-contained: hardcodes shapes from the problem spec.

  g_seq:              [4, 128, 512]  f32
  latents_logits_seq: [4, 128, 32, 32] f32
  output:             scalar f32

Math (per batch element b, over the P = S*(S-1)/2 unordered pairs):
  dist[i,j] = || g[i] - g[j] ||
  jsd[i,j]  = (1/(2L)) * ( LA_i + LA_j - sum_{l,v} s*log(0.5*s) ),
              s = p_i + p_j,  LA_i = sum_{l,v} p_i * log p_i
  loss = ALPHA * mean( exp(-jsd^2/(2 sigma^2)) * (jsd - dist)^2 )
         + (1-ALPHA) * mean_{b,s}( -sum_d g )

Sharding: 8 cores = 4 batch elements x 2 pair-parity groups. Core (b, par)
receives g/logits with the sequence dim permuted so that its 64 "i" rows sit
in positions 0..63; it computes, for each local i in [0,64), the pair terms
against all j in [i&~1, 128) (a uniform triangular window — identical program
on every core). A host-supplied {0,1} mask keeps exactly the pairs whose
global min-index parity matches the core, so each unordered pair is counted
exactly once across the 2 cores of a batch element. Each core returns partial
sums; the host reduces them to the scalar loss.

Device layout for the heavy pairwise-JSD loop: probabilities are transposed
to pT[lv, j] (partitions = latent dim, free = sequence). Then for a fixed i,
s = p_j + p_i is a free-dim broadcast add on DVE, t = log(0.5*s) is one ACT
op (the 0.5 comes free via ACT's input scale), u = s*t on DVE, and the
partition-direction reduction sum_lv u is a ones-vector matmul on the
otherwise idle TensorE, accumulated in PSUM.
"""

from contextlib import ExitStack

import numpy as np

import concourse.bacc as bacc
import concourse.bass as bass
import concourse.mybir as mybir
import concourse.tile as tile
from concourse.bass_utils import run_bass_kernel_spmd
from concourse.masks import make_identity

AF = mybir.ActivationFunctionType
AX = mybir.AxisListType
OP = mybir.AluOpType
F32 = mybir.dt.float32
BF16 = mybir.dt.bfloat16

ALPHA = 0.54
SIGMA = 1.2
B = 4
S = 128
L = 32
V = 32
LV = L * V
DG = 512
NT = LV // 128  # lv partition-tiles
NI = 64  # local i rows per core
N_CORES = 8
P_PAIRS = S * (S - 1) // 2


# ---------------------------------------------------------------- device code
def _program(ctx: ExitStack, tc: "tile.TileContext", logits, g, mask, out,
             stage: str = "full"):
    nc = tc.nc
    P = 128

    def _bail(*tiles):
        """Debug early-exit: write something derived from live tiles."""
        pool = ctx.enter_context(tc.tile_pool(name="bail", bufs=1))
        ob = pool.tile([P, 2], F32, name="bail_out")
        nc.vector.memset(ob[:], 1.0)
        nc.sync.dma_start(out[:], ob[:])
        return True

    const = ctx.enter_context(tc.tile_pool(name="const", bufs=1))
    pers = ctx.enter_context(tc.tile_pool(name="pers", bufs=1))
    work = ctx.enter_context(tc.tile_pool(name="work", bufs=2))
    psum = ctx.enter_context(tc.tile_pool(name="psum", bufs=2, space=bass.MemorySpace.PSUM))
    cpsum = ctx.enter_context(tc.tile_pool(name="cpsum", bufs=2, space=bass.MemorySpace.PSUM))
    mpool = ctx.enter_context(tc.tile_pool(name="mpool", bufs=3))

    # ---------------- loads ----------------
    logits_sb = pers.tile([P, LV], F32, name="logits_sb")
    nc.sync.dma_start(logits_sb[:], logits.rearrange("s l v -> s (l v)"))
    g_sb = pers.tile([P, DG], F32, name="g_sb")
    nc.sync.dma_start(g_sb[:], g[:])
    mask_sb = pers.tile([NI, S], mybir.dt.uint8, name="mask_sb")
    nc.sync.dma_start(mask_sb[:], mask[:])

    ident = const.tile([P, P], F32, name="ident")
    make_identity(nc, ident[:])
    ones_bf = const.tile([P, 1], BF16, name="ones_bf")
    nc.gpsimd.memset(ones_bf[:], 1.0)
    zeros_bf = const.tile([P, P], BF16, name="zeros_bf")
    nc.gpsimd.memset(zeros_bf[:], 0.0)

    if stage == "loads":
        return _bail()

    def lv3(ap):
        return ap.rearrange("p (l v) -> p l v", v=V)

    # ---------------- softmax / entropy prologue ----------------
    mx = work.tile([P, L], F32, name="mx")
    nc.vector.reduce_max(mx[:], lv3(logits_sb[:]), axis=AX.X)
    if stage == "p1":
        return _bail()
    z = pers.tile([P, LV], F32, name="z")  # z = logits - max  (later: logp)
    nc.vector.tensor_sub(
        lv3(z[:]), lv3(logits_sb[:]), mx[:].unsqueeze(2).to_broadcast((P, L, V))
    )
    if stage == "p2":
        return _bail()
    e = pers.tile([P, LV], F32, name="e")
    nc.scalar.activation(e[:], z[:], AF.Exp)
    if stage == "p3":
        return _bail()
    ssum = work.tile([P, L], F32, name="ssum")
    nc.vector.reduce_sum(ssum[:], lv3(e[:]), axis=AX.X)
    rs = work.tile([P, L], F32, name="rs")
    nc.vector.reciprocal(rs[:], ssum[:])
    p32 = pers.tile([P, LV], F32, name="p32")
    nc.vector.tensor_mul(
        lv3(p32[:]), lv3(e[:]), rs[:].unsqueeze(2).to_broadcast((P, L, V))
    )
    if stage == "p4":
        return _bail()
    logS = work.tile([P, L], F32, name="logS")
    nc.scalar.activation(logS[:], ssum[:], AF.Ln)
    # z <- logp = z - log(sum)
    nc.vector.tensor_sub(
        lv3(z[:]), lv3(z[:]), logS[:].unsqueeze(2).to_broadcast((P, L, V))
    )

    if stage == "p5":
        return _bail()
    # LA[i] = sum_{l,v} p * logp   (f32 accumulate)
    scr = pers.tile([P, LV], F32, name="scr")
    LA = pers.tile([P, 1], F32, name="LA")
    nc.vector.tensor_mul(scr[:], p32[:], z[:])
    nc.vector.reduce_sum(LA[:], scr[:], axis=AX.X)
    # row norms n[i] = sum_d g^2 ; gsum[i] = sum_d g
    nvec = pers.tile([P, 1], F32, name="nvec")
    nc.vector.tensor_mul(scr[:, 0:DG], g_sb[:], g_sb[:])
    nc.vector.reduce_sum(nvec[:], scr[:, 0:DG], axis=AX.X)
    if stage == "p6":
        return _bail()
    gsum = pers.tile([P, 1], F32, name="gsum")
    nc.vector.reduce_sum(gsum[:], g_sb[:], axis=AX.X)

    if stage == "prologue":
        return _bail()

    # ---------------- transposes ----------------
    # pT[lv % 128, k, s] = p[s, 128k + lv%128], bf16
    pT = pers.tile([P, NT, P], BF16, name="pT")
    for k in range(NT):
        tp = psum.tile([P, P], F32, name="tp", tag="tp")
        nc.tensor.transpose(tp[:], p32[:, k * P:(k + 1) * P], ident[:])
        nc.vector.tensor_copy(pT[:, k, :], tp[:])

    # gT chunks (f32) and -2*gT for the gram matmul
    gT = pers.tile([P, 4, P], F32, name="gT")
    gTn = pers.tile([P, 4, P], F32, name="gTn")
    for k in range(4):
        tp = psum.tile([P, P], F32, name="tp2", tag="tp")
        nc.tensor.transpose(tp[:], g_sb[:, k * P:(k + 1) * P], ident[:])
        nc.vector.tensor_copy(gT[:, k, :], tp[:])
        nc.vector.tensor_scalar_mul(gTn[:, k, :], gT[:, k, :], -2.0)

    # -2 * G[i, j] for local i rows 0..63, all j  (accumulate over 4 K-chunks)
    G2p = psum.tile([NI, P], F32, name="G2p", tag="g2")
    for k in range(4):
        nc.tensor.matmul(G2p[:], gT[:, k, 0:NI], gTn[:, k, :],
                         start=(k == 0), stop=(k == 3))

    # LA / n as free-axis rows, replicated to [NI, P] via a DRAM bounce
    rowsrc = work.tile([P, 2], F32, name="rowsrc")
    nc.vector.tensor_copy(rowsrc[:, 0:1], LA[:])
    nc.vector.tensor_copy(rowsrc[:, 1:2], nvec[:])
    rp = psum.tile([2, P], F32, name="rp", tag="tp")
    nc.tensor.transpose(rp[:], rowsrc[:], ident[:])
    rows_sb = pers.tile([2, P], F32, name="rows_sb")
    nc.vector.tensor_copy(rows_sb[:], rp[:])
    rows_dram = nc.dram_tensor("rows_bounce", [2, P], F32)
    nc.sync.dma_start(rows_dram[:], rows_sb[:])
    LAT_rep = pers.tile([NI, P], F32, name="LAT_rep")
    nc.sync.dma_start(LAT_rep[:], rows_dram[0:1, :].to_broadcast((NI, P)))
    nT_rep = pers.tile([NI, P], F32, name="nT_rep")
    nc.sync.dma_start(nT_rep[:], rows_dram[1:2, :].to_broadcast((NI, P)))

    if stage == "transposes":
        return _bail()

    # ---------------- main pairwise loop ----------------
    # csum[i, j] = sum_lv (p_i + p_j) * log(0.5*(p_i + p_j))
    # The ones-matmul reduces over partitions into a PSUM row per i; groups of
    # GRP rows share a persistent PSUM tile (memset once — later groups may
    # leave stale-but-finite values in columns outside their window, which the
    # mask kills). ACT copies each full group row to SBUF, then a DMA reshapes
    # the row-major group to one-row-per-partition in csum_sb.
    GRP = 8
    csum_sb = pers.tile([NI, P], F32, name="csum_sb")
    cps = None
    n_i = 8 if stage == "main8" else NI
    for i in range(n_i):
        jr = i & ~1
        fd = P - jr
        s_t = mpool.tile([P, NT, fd], BF16, name=f"s_{i}", tag="s")
        nc.vector.tensor_add(
            s_t[:], pT[:, :, jr:P],
            pT[:, :, i:i + 1].to_broadcast((P, NT, fd)),
        )
        t_t = mpool.tile([P, NT, fd], BF16, name=f"t_{i}", tag="t")
        nc.scalar.activation(t_t[:], s_t[:], AF.Ln, scale=0.5)
        u_t = mpool.tile([P, NT, fd], BF16, name=f"u_{i}", tag="u")
        nc.vector.tensor_mul(u_t[:], s_t[:], t_t[:])
        if i % GRP == 0:
            cps = cpsum.tile([1, GRP, P], F32, name=f"cps_{i}", tag="cps")
        # zero-fill the full row first so columns outside the window are
        # initialized, then accumulate the 8 lv-chunk reductions on top.
        nc.tensor.matmul(cps[0:1, i % GRP, 0:P], ones_bf[:], zeros_bf[:, 0:P],
                         start=True, stop=False)
        for k in range(NT):
            nc.tensor.matmul(
                cps[0:1, i % GRP, jr:P], ones_bf[:], u_t[:, k, :],
                start=False, stop=(k == NT - 1),
            )
        if i % GRP == GRP - 1:
            rowtmp = mpool.tile([1, GRP, P], F32, name=f"rowtmp_{i}", tag="rowtmp")
            nc.scalar.copy(rowtmp[:], cps[:])
            nc.sync.dma_start(csum_sb[i - (GRP - 1):i + 1, :], rowtmp[0:1, :, :])

    if stage in ("main8", "mainloop"):
        return _bail()

    # ---------------- epilogue ----------------
    # dist^2 = n_i + n_j - 2 G_ij   (clamped), dist = sqrt
    d2 = pers.tile([NI, P], F32, name="d2")
    nc.vector.tensor_add(d2[:], G2p[:], nT_rep[:])
    nc.vector.tensor_scalar_add(d2[:], d2[:], nvec[0:NI, :])
    nc.vector.tensor_scalar_max(d2[:], d2[:], 0.0)

    # jsd = (LA_i + LA_j - csum) / (2L)
    jsd = pers.tile([NI, P], F32, name="jsd")
    nc.vector.tensor_scalar_add(jsd[:], LAT_rep[:], LA[0:NI, :])
    nc.vector.tensor_sub(jsd[:], jsd[:], csum_sb[:])
    nc.vector.tensor_scalar_mul(jsd[:], jsd[:], 1.0 / (2.0 * L))

    j2 = work.tile([NI, P], F32, name="j2")
    nc.vector.tensor_mul(j2[:], jsd[:], jsd[:])
    w = pers.tile([NI, P], F32, name="w")
    nc.scalar.activation(w[:], j2[:], AF.Exp, scale=-1.0 / (2.0 * SIGMA * SIGMA))

    dist = pers.tile([NI, P], F32, name="dist")
    nc.scalar.activation(dist[:], d2[:], AF.Sqrt)

    diff = work.tile([NI, P], F32, name="diff")
    nc.vector.tensor_sub(diff[:], jsd[:], dist[:])
    nc.vector.tensor_mul(diff[:], diff[:], diff[:])
    nc.vector.tensor_mul(diff[:], diff[:], w[:])
    # select-by-mask (not multiply): masked-out columns can hold stale-PSUM
    # garbage (possibly NaN) which 0*x would propagate.
    contrib = pers.tile([NI, P], F32, name="contrib")
    nc.vector.memset(contrib[:], 0.0)
    nc.vector.copy_predicated(contrib[:], mask_sb[:], diff[:])
    rowsum = work.tile([NI, 1], F32, name="rowsum")
    nc.vector.reduce_sum(rowsum[:], contrib[:], axis=AX.X)

    out_sb = pers.tile([P, 2], F32, name="out_sb")
    nc.vector.memset(out_sb[:], 0.0)
    nc.vector.tensor_copy(out_sb[0:NI, 0:1], rowsum[:])
    nc.vector.tensor_copy(out_sb[:, 1:2], gsum[:])
    nc.sync.dma_start(out[:], out_sb[:])


_NC_CACHE = {}


def _get_nc(stage: str = "full"):
    if stage not in _NC_CACHE:
        nc = bacc.Bacc("TRN2", target_bir_lowering=False, debug=False,
                       num_devices=N_CORES)
        logits_d = nc.dram_tensor("logits", [S, L, V], F32, kind="ExternalInput")
        g_d = nc.dram_tensor("g", [S, DG], F32, kind="ExternalInput")
        mask_d = nc.dram_tensor("mask", [NI, S], mybir.dt.uint8, kind="ExternalInput")
        out_d = nc.dram_tensor("out", [S, 2], F32, kind="ExternalOutput")
        with tile.TileContext(nc) as tc, ExitStack() as ctx:
            _program(ctx, tc, logits_d.ap(), g_d.ap(), mask_d.ap(), out_d.ap(),
                     stage=stage)
        nc.compile()
        _NC_CACHE[stage] = nc
    return _NC_CACHE[stage]


# ---------------------------------------------------------------- host side
def _perm(parity: int) -> np.ndarray:
    if parity == 0:
        return np.concatenate([np.arange(0, S, 2), np.arange(1, S, 2)])
    return np.concatenate([np.arange(1, S, 2), np.arange(0, S, 2)])


def _mask_np(parity: int) -> np.ndarray:
    pm = _perm(parity)
    m = (pm[None, :S] > pm[:NI, None]).astype(np.uint8)
    return np.ascontiguousarray(m)


def make_in_maps(g_seq: np.ndarray, latents_logits_seq: np.ndarray):
    in_maps = []
    for c in range(N_CORES):
        b, par = c >> 1, c & 1
        pm = _perm(par)
        in_maps.append({
            "logits": np.ascontiguousarray(
                np.asarray(latents_logits_seq, np.float32)[b][pm]),
            "g": np.ascontiguousarray(np.asarray(g_seq, np.float32)[b][pm]),
            "mask": _mask_np(par),
        })
    return in_maps


def combine_outputs(outs) -> np.float32:
    dist_sum = 0.0
    cap_sum = 0.0
    for c in range(N_CORES):
        o = np.asarray(outs[c]["out"], np.float64)
        dist_sum += o[:NI, 0].sum()
        if c % 2 == 0:
            cap_sum += o[:, 1].sum()
    loss_dist = dist_sum / (B * P_PAIRS)
    loss_cap = -cap_sum / (B * S)
    return np.float32(ALPHA * loss_dist + (1.0 - ALPHA) * loss_cap)


def kernel(g_seq: np.ndarray, latents_logits_seq: np.ndarray) -> np.ndarray:
    nc = _get_nc()
    in_maps = make_in_maps(g_seq, latents_logits_seq)
    res = run_bass_kernel_spmd(nc, in_maps, list(range(N_CORES))).results
    return combine_outputs(res)
